# revision 1
# baseline (speedup 1.0000x reference)
"""Trainium2 Bass kernel for an 8-layer dense MLP (nn_FCN).

Reference computation (fp32):
    y0 = x                                  [4096, 2048]
    y_{l+1} = relu((y_l @ W_l.T) / sqrt(2048))   W: [8, 2048, 2048]
    out = y_8 @ beta / 2048                 beta: [2048, 1024] -> out [4096, 1024]

Strategy: data-parallel over batch across 8 NeuronCores (512 rows each);
weights/beta replicated. On-chip layout is channel-major ("transposed"):
activations live in SBUF as [128(part)=ch, 16(k-tile), 512(batch)], so each
layer is out[o, b] = sum_i WT[i, o] * Y[i, b] with the contraction dim on
partitions, and layer outputs land back in the same channel-major layout —
no transposes anywhere on device. W is pre-transposed once on the host
(WT[l] = W[l].T, contiguous); beta is already [h, out] = lhsT layout.

Matmuls run as float32r (TF32-like, 11-bit mantissa, full PE issue rate at
N=512) accumulating fp32 in PSUM; the ReLU epilogue runs on the scalar
engine reading PSUM and writing the next layer's fp32r activation tile.
"""

import math
from contextlib import ExitStack

import numpy as np

P = 128
H = 2048
OUT = 1024
B_TOTAL = 4096
N_CORES = 8
B = B_TOTAL // N_CORES  # 512 batch rows per core
L = 8
KI = H // P  # 16 contraction tiles per matmul
O_CHUNK = 512  # output channels per weight DMA chunk
SCALE = 1.0 / math.sqrt(H)
OUT_SCALE = 1.0 / H

_BUILD_CACHE = {}


def _build(repeat=1, loop=0):
    """loop>0 wraps `repeat` forward passes in an on-device For_i loop of
    `loop` iterations (timing tool only; grading path uses repeat=1, loop=0)."""
    key = (repeat, loop)
    if key in _BUILD_CACHE:
        return _BUILD_CACHE[key]

    import concourse.mybir as mybir
    import concourse.tile as tile
    from concourse import bacc

    f32 = mybir.dt.float32
    f32r = mybir.dt.float32r

    nc = bacc.Bacc("TRN2", target_bir_lowering=False, debug=False)
    xt_d = nc.dram_tensor("xt", [H, B], f32r, kind="ExternalInput").ap()
    wt_d = nc.dram_tensor("wt", [L, H, H], f32r, kind="ExternalInput").ap()
    beta_d = nc.dram_tensor("beta", [H, OUT], f32r, kind="ExternalInput").ap()
    out_d = nc.dram_tensor("out", [OUT, B], f32, kind="ExternalOutput").ap()

    xt_t = xt_d.rearrange("(k p) b -> p k b", p=P)
    out_t = out_d.rearrange("(k p) b -> p k b", p=P)
    beta_t = beta_d.rearrange("(k p) o -> p k o", p=P)

    with tile.TileContext(nc) as tc, ExitStack() as ctx:
        ypool = ctx.enter_context(tc.tile_pool(name="y", bufs=2))
        wpool = ctx.enter_context(tc.tile_pool(name="w", bufs=3))
        opool = ctx.enter_context(tc.tile_pool(name="o", bufs=4))
        pspool = ctx.enter_context(tc.tile_pool(name="ps", bufs=8, space="PSUM"))

        # Warm the PE (HAM clock-gate releases after ~3.4us of sustained
        # activity) with dummy matmuls on zeroed tiles while the first x and
        # weight DMAs are in flight; the array hits 2.4GHz before real work.
        warm_w = opool.tile([P, P], f32r, tag="warmw")
        warm_y = opool.tile([P, B], f32r, tag="warmy")
        nc.sync.dma_start(warm_w[:], beta_t[:, 0, 0:P])
        nc.sync.dma_start(warm_y[:], xt_t[:, 0, :])
        # 8 cold-rate dummies ~= 3.4us: exactly one HAM window, finishing as
        # the first (split) weight k-group lands, so real MMs start warm.
        for _d in range(8):
            ps = pspool.tile([P, B], f32, tag="ps")
            nc.tensor.matmul(
                ps[:], lhsT=warm_w[:], rhs=warm_y[:], start=True, stop=True
            )

        if loop:
            loop_cm = tc.For_i(0, loop, 1)
            loop_cm.__enter__()

        for _ in range(repeat):
            y_cur = ypool.tile([P, KI, B], f32r, tag="y")
            for kg in range(4):
                nc.sync.dma_start(
                    y_cur[:, kg * 4 : (kg + 1) * 4, :],
                    xt_t[:, kg * 4 : (kg + 1) * 4, :],
                )

            for layer in range(L):
                wt_l = wt_d[layer].rearrange("(k p) o -> p k o", p=P)
                y_next = ypool.tile([P, KI, B], f32r, tag="y")
                for oc in range(H // O_CHUNK):
                    w_sb = wpool.tile([P, KI, O_CHUNK], f32r, tag="w")
                    if layer == 0 and oc == 0:
                        # split the very first chunk by k-groups so layer-0
                        # matmuls start after ~1MB instead of the full 4MB
                        for kg in range(4):
                            nc.sync.dma_start(
                                w_sb[:, kg * 4 : (kg + 1) * 4, :],
                                wt_l[:, kg * 4 : (kg + 1) * 4, 0:O_CHUNK],
                            )
                    else:
                        nc.sync.dma_start(
                            w_sb[:], wt_l[:, :, oc * O_CHUNK : (oc + 1) * O_CHUNK]
                        )
                    for os_ in range(O_CHUNK // P):
                        ps = pspool.tile([P, B], f32, tag="ps")
                        for ki in range(KI):
                            nc.tensor.matmul(
                                ps[:],
                                lhsT=w_sb[:, ki, os_ * P : (os_ + 1) * P],
                                rhs=y_cur[:, ki, :],
                                start=(ki == 0),
                                stop=(ki == KI - 1),
                            )
                        ot = oc * (O_CHUNK // P) + os_
                        nc.scalar.activation(
                            y_next[:, ot, :],
                            ps[:],
                            mybir.ActivationFunctionType.Relu,
                            scale=SCALE,
                        )
                y_cur = y_next

            # readout: out[o, b] = sum_h beta[h, o] * y[h, b], scaled by 1/H
            for oc in range(OUT // O_CHUNK):
                b_sb = wpool.tile([P, KI, O_CHUNK], f32r, tag="w")
                nc.sync.dma_start(
                    b_sb[:], beta_t[:, :, oc * O_CHUNK : (oc + 1) * O_CHUNK]
                )
                for os_ in range(O_CHUNK // P):
                    ps = pspool.tile([P, B], f32, tag="ps")
                    for ki in range(KI):
                        nc.tensor.matmul(
                            ps[:],
                            lhsT=b_sb[:, ki, os_ * P : (os_ + 1) * P],
                            rhs=y_cur[:, ki, :],
                            start=(ki == 0),
                            stop=(ki == KI - 1),
                        )
                    o_sb = opool.tile([P, B], f32, tag="o")
                    nc.scalar.activation(
                        o_sb[:],
                        ps[:],
                        mybir.ActivationFunctionType.Copy,
                        scale=OUT_SCALE,
                    )
                    nc.sync.dma_start(
                        out_t[:, oc * (O_CHUNK // P) + os_, :], o_sb[:]
                    )

        if loop:
            loop_cm.__exit__(None, None, None)

    nc.compile()
    _BUILD_CACHE[key] = nc
    return nc


def _prep_in_maps(x, W, beta):
    x = np.asarray(x, dtype=np.float32)
    W = np.asarray(W, dtype=np.float32)
    beta = np.asarray(beta, dtype=np.float32)
    xt = np.ascontiguousarray(x.T)  # [H, B_TOTAL]
    wt = np.ascontiguousarray(W.transpose(0, 2, 1))  # [L, H(i), H(o)]
    beta = np.ascontiguousarray(beta)
    return [
        {"xt": np.ascontiguousarray(xt[:, c * B : (c + 1) * B]), "wt": wt, "beta": beta}
        for c in range(N_CORES)
    ]


def kernel(x, W, beta):
    from concourse.bass_utils import run_bass_kernel_spmd

    nc = _build()
    in_maps = _prep_in_maps(x, W, beta)
    res = run_bass_kernel_spmd(nc, in_maps, core_ids=list(range(N_CORES)))
    outs = [r["out"] for r in res.results]  # each [OUT, B] channel-major
    return np.concatenate([o.T for o in outs], axis=0).astype(np.float32)



# revision 3
# speedup vs baseline: 1.0410x; 1.0410x over previous
"""Trainium2 Bass kernel for an 8-layer dense MLP (nn_FCN).

Reference computation (fp32):
    y0 = x                                  [4096, 2048]
    y_{l+1} = relu((y_l @ W_l.T) / sqrt(2048))   W: [8, 2048, 2048]
    out = y_8 @ beta / 2048                 beta: [2048, 1024] -> out [4096, 1024]

Strategy: data-parallel over batch across 8 NeuronCores (512 rows each);
weights/beta replicated. On-chip layout is channel-major ("transposed"):
activations live in SBUF as [128(part)=ch, 16(k-tile), 512(batch)], so each
layer is out[o, b] = sum_i WT[i, o] * Y[i, b] with the contraction dim on
partitions, and layer outputs land back in the same channel-major layout —
no transposes anywhere on device. W is pre-transposed once on the host
(WT[l] = W[l].T, contiguous); beta is already [h, out] = lhsT layout.

Matmuls run as float32r (TF32-like, 11-bit mantissa, full PE issue rate at
N=512) accumulating fp32 in PSUM; the ReLU epilogue runs on the scalar
engine reading PSUM and writing the next layer's fp32r activation tile.
"""

import math
from contextlib import ExitStack

import numpy as np

P = 128
H = 2048
OUT = 1024
B_TOTAL = 4096
N_CORES = 8
B = B_TOTAL // N_CORES  # 512 batch rows per core
L = 8
KI = H // P  # 16 contraction tiles per matmul
O_CHUNK = 512  # output channels per weight DMA chunk
SCALE = 1.0 / math.sqrt(H)
OUT_SCALE = 1.0 / H

_BUILD_CACHE = {}


def _build(repeat=1, loop=0):
    """loop>0 wraps `repeat` forward passes in an on-device For_i loop of
    `loop` iterations (timing tool only; grading path uses repeat=1, loop=0)."""
    key = (repeat, loop)
    if key in _BUILD_CACHE:
        return _BUILD_CACHE[key]

    import concourse.mybir as mybir
    import concourse.tile as tile
    from concourse import bacc

    f32 = mybir.dt.float32
    f16 = mybir.dt.float16

    nc = bacc.Bacc("TRN2", target_bir_lowering=False, debug=False)
    xt_d = nc.dram_tensor("xt", [H, B], f16, kind="ExternalInput").ap()
    wt_d = nc.dram_tensor("wt", [L, H, H], f16, kind="ExternalInput").ap()
    beta_d = nc.dram_tensor("beta", [H, OUT], f16, kind="ExternalInput").ap()
    out_d = nc.dram_tensor("out", [OUT, B], f32, kind="ExternalOutput").ap()

    xt_t = xt_d.rearrange("(k p) b -> p k b", p=P)
    out_t = out_d.rearrange("(k p) b -> p k b", p=P)
    beta_t = beta_d.rearrange("(k p) o -> p k o", p=P)

    with tile.TileContext(nc) as tc, ExitStack() as ctx:
        ypool = ctx.enter_context(tc.tile_pool(name="y", bufs=2))
        wpool = ctx.enter_context(tc.tile_pool(name="w", bufs=3))
        opool = ctx.enter_context(tc.tile_pool(name="o", bufs=4))
        pspool = ctx.enter_context(tc.tile_pool(name="ps", bufs=8, space="PSUM"))

        # Warm the PE (HAM clock-gate releases after ~3.4us of sustained
        # activity) with dummy matmuls on zeroed tiles while the first x and
        # weight DMAs are in flight; the array hits 2.4GHz before real work.
        warm_w = opool.tile([P, P], f16, tag="warmw")
        warm_y = opool.tile([P, B], f16, tag="warmy")
        nc.sync.dma_start(warm_w[:], beta_t[:, 0, 0:P])
        nc.sync.dma_start(warm_y[:], xt_t[:, 0, :])
        # 8 cold-rate dummies ~= 3.4us: exactly one HAM window, finishing as
        # the first (split) weight k-group lands, so real MMs start warm.
        for _d in range(8):
            ps = pspool.tile([P, B], f32, tag="ps")
            nc.tensor.matmul(
                ps[:], lhsT=warm_w[:], rhs=warm_y[:], start=True, stop=True
            )

        if loop:
            loop_cm = tc.For_i(0, loop, 1)
            loop_cm.__enter__()

        for _ in range(repeat):
            y_cur = ypool.tile([P, KI, B], f16, tag="y")
            for kg in range(4):
                nc.sync.dma_start(
                    y_cur[:, kg * 4 : (kg + 1) * 4, :],
                    xt_t[:, kg * 4 : (kg + 1) * 4, :],
                )

            for layer in range(L):
                wt_l = wt_d[layer].rearrange("(k p) o -> p k o", p=P)
                y_next = ypool.tile([P, KI, B], f16, tag="y")
                for oc in range(H // O_CHUNK):
                    w_sb = wpool.tile([P, KI, O_CHUNK], f16, tag="w")
                    if layer == 0 and oc == 0:
                        # split the very first chunk by k-groups so layer-0
                        # matmuls start after ~1MB instead of the full 4MB
                        for kg in range(4):
                            nc.sync.dma_start(
                                w_sb[:, kg * 4 : (kg + 1) * 4, :],
                                wt_l[:, kg * 4 : (kg + 1) * 4, 0:O_CHUNK],
                            )
                    else:
                        nc.sync.dma_start(
                            w_sb[:], wt_l[:, :, oc * O_CHUNK : (oc + 1) * O_CHUNK]
                        )
                    for os_ in range(O_CHUNK // P):
                        ps = pspool.tile([P, B], f32, tag="ps")
                        for ki in range(KI):
                            nc.tensor.matmul(
                                ps[:],
                                lhsT=w_sb[:, ki, os_ * P : (os_ + 1) * P],
                                rhs=y_cur[:, ki, :],
                                start=(ki == 0),
                                stop=(ki == KI - 1),
                            )
                        ot = oc * (O_CHUNK // P) + os_
                        nc.scalar.activation(
                            y_next[:, ot, :],
                            ps[:],
                            mybir.ActivationFunctionType.Relu,
                            scale=SCALE,
                        )
                y_cur = y_next

            # readout: out[o, b] = sum_h beta[h, o] * y[h, b], scaled by 1/H
            for oc in range(OUT // O_CHUNK):
                b_sb = wpool.tile([P, KI, O_CHUNK], f16, tag="w")
                nc.sync.dma_start(
                    b_sb[:], beta_t[:, :, oc * O_CHUNK : (oc + 1) * O_CHUNK]
                )
                for os_ in range(O_CHUNK // P):
                    ps = pspool.tile([P, B], f32, tag="ps")
                    for ki in range(KI):
                        nc.tensor.matmul(
                            ps[:],
                            lhsT=b_sb[:, ki, os_ * P : (os_ + 1) * P],
                            rhs=y_cur[:, ki, :],
                            start=(ki == 0),
                            stop=(ki == KI - 1),
                        )
                    o_sb = opool.tile([P, B], f32, tag="o")
                    nc.scalar.activation(
                        o_sb[:],
                        ps[:],
                        mybir.ActivationFunctionType.Copy,
                        scale=OUT_SCALE,
                    )
                    nc.sync.dma_start(
                        out_t[:, oc * (O_CHUNK // P) + os_, :], o_sb[:]
                    )

        if loop:
            loop_cm.__exit__(None, None, None)

    nc.compile()
    _BUILD_CACHE[key] = nc
    return nc


def _prep_in_maps(x, W, beta):
    x = np.asarray(x, dtype=np.float32)
    W = np.asarray(W, dtype=np.float32)
    beta = np.asarray(beta, dtype=np.float32)
    xt = np.ascontiguousarray(x.T.astype(np.float16))  # [H, B_TOTAL]
    wt = np.ascontiguousarray(W.transpose(0, 2, 1).astype(np.float16))  # [L, H(i), H(o)]
    beta = np.ascontiguousarray(beta.astype(np.float16))
    return [
        {"xt": np.ascontiguousarray(xt[:, c * B : (c + 1) * B]), "wt": wt, "beta": beta}
        for c in range(N_CORES)
    ]


def kernel(x, W, beta):
    from concourse.bass_utils import run_bass_kernel_spmd

    nc = _build()
    in_maps = _prep_in_maps(x, W, beta)
    res = run_bass_kernel_spmd(nc, in_maps, core_ids=list(range(N_CORES)))
    outs = [r["out"] for r in res.results]  # each [OUT, B] channel-major
    return np.concatenate([o.T for o in outs], axis=0).astype(np.float32)



# revision 6
# speedup vs baseline: 1.0726x; 1.0304x over previous
"""Trainium2 Bass kernel for an 8-layer dense MLP (nn_FCN).

Reference computation (fp32):
    y0 = x                                  [4096, 2048]
    y_{l+1} = relu((y_l @ W_l.T) / sqrt(2048))   W: [8, 2048, 2048]
    out = y_8 @ beta / 2048                 beta: [2048, 1024] -> out [4096, 1024]

Strategy: data-parallel over batch across 8 NeuronCores (512 rows each);
weights/beta replicated. On-chip layout is channel-major ("transposed"):
activations live in SBUF as [128(part)=ch, 16(k-tile), 512(batch)], so each
layer is out[o, b] = sum_i WT[i, o] * Y[i, b] with the contraction dim on
partitions, and layer outputs land back in the same channel-major layout —
no transposes anywhere on device. W is pre-transposed once on the host
(WT[l] = W[l].T, contiguous); beta is already [h, out] = lhsT layout.

Matmuls run as float32r (TF32-like, 11-bit mantissa, full PE issue rate at
N=512) accumulating fp32 in PSUM; the ReLU epilogue runs on the scalar
engine reading PSUM and writing the next layer's fp32r activation tile.
"""

import math
from contextlib import ExitStack

import numpy as np

P = 128
H = 2048
OUT = 1024
B_TOTAL = 4096
N_CORES = 8
B = B_TOTAL // N_CORES  # 512 batch rows per core
L = 8
KI = H // P  # 16 contraction tiles per matmul
O_CHUNK = 512  # output channels per weight DMA chunk
SCALE = 1.0 / math.sqrt(H)
OUT_SCALE = 1.0 / H

_BUILD_CACHE = {}


def _build(repeat=1, loop=0):
    """loop>0 wraps `repeat` forward passes in an on-device For_i loop of
    `loop` iterations (timing tool only; grading path uses repeat=1, loop=0)."""
    key = (repeat, loop)
    if key in _BUILD_CACHE:
        return _BUILD_CACHE[key]

    import concourse.mybir as mybir
    import concourse.tile as tile
    from concourse import bacc

    f32 = mybir.dt.float32
    f16 = mybir.dt.bfloat16

    nc = bacc.Bacc("TRN2", target_bir_lowering=False, debug=False)
    xt_d = nc.dram_tensor("xt", [H, B], f16, kind="ExternalInput").ap()
    wt_d = nc.dram_tensor("wt", [L, H, H], f16, kind="ExternalInput").ap()
    beta_d = nc.dram_tensor("beta", [H, OUT], f16, kind="ExternalInput").ap()
    out_d = nc.dram_tensor("out", [OUT, B], f32, kind="ExternalOutput").ap()

    xt_t = xt_d.rearrange("(k p) b -> p k b", p=P)
    out_t = out_d.rearrange("(k p) b -> p k b", p=P)
    beta_t = beta_d.rearrange("(k p) o -> p k o", p=P)

    with tile.TileContext(nc) as tc, ExitStack() as ctx:
        ypool = ctx.enter_context(tc.tile_pool(name="y", bufs=2))
        wpool = ctx.enter_context(tc.tile_pool(name="w", bufs=3))
        opool = ctx.enter_context(tc.tile_pool(name="o", bufs=4))
        pspool = ctx.enter_context(tc.tile_pool(name="ps", bufs=8, space="PSUM"))

        # Warm the PE (HAM clock-gate releases after ~3.4us of sustained
        # activity) with dummy matmuls on zeroed tiles while the first x and
        # weight DMAs are in flight; the array hits 2.4GHz before real work.
        warm_w = opool.tile([P, P], f16, tag="warmw")
        warm_y = opool.tile([P, B], f16, tag="warmy")
        nc.sync.dma_start(warm_w[:], beta_t[:, 0, 0:P])
        nc.sync.dma_start(warm_y[:], xt_t[:, 0, :])
        # 8 cold-rate dummies ~= 3.4us: exactly one HAM window, finishing as
        # the first (split) weight k-group lands, so real MMs start warm.
        for _d in range(8):
            ps = pspool.tile([P, B], f32, tag="ps")
            nc.tensor.matmul(
                ps[:], lhsT=warm_w[:], rhs=warm_y[:], start=True, stop=True
            )

        if loop:
            loop_cm = tc.For_i(0, loop, 1)
            loop_cm.__enter__()

        for _ in range(repeat):
            y_cur = ypool.tile([P, KI, B], f16, tag="y")
            for kg in range(4):
                nc.sync.dma_start(
                    y_cur[:, kg * 4 : (kg + 1) * 4, :],
                    xt_t[:, kg * 4 : (kg + 1) * 4, :],
                )

            for layer in range(L):
                wt_l = wt_d[layer].rearrange("(k p) o -> p k o", p=P)
                y_next = ypool.tile([P, KI, B], f16, tag="y")
                for oc in range(H // O_CHUNK):
                    w_sb = wpool.tile([P, KI, O_CHUNK], f16, tag="w")
                    if layer == 0 and oc == 0:
                        # split the very first chunk by k-groups so layer-0
                        # matmuls start after ~1MB instead of the full 4MB
                        for kg in range(4):
                            nc.sync.dma_start(
                                w_sb[:, kg * 4 : (kg + 1) * 4, :],
                                wt_l[:, kg * 4 : (kg + 1) * 4, 0:O_CHUNK],
                            )
                    else:
                        nc.sync.dma_start(
                            w_sb[:], wt_l[:, :, oc * O_CHUNK : (oc + 1) * O_CHUNK]
                        )
                    for os_ in range(O_CHUNK // P):
                        ps = pspool.tile([P, B], f32, tag="ps")
                        for ki in range(KI):
                            nc.tensor.matmul(
                                ps[:],
                                lhsT=w_sb[:, ki, os_ * P : (os_ + 1) * P],
                                rhs=y_cur[:, ki, :],
                                start=(ki == 0),
                                stop=(ki == KI - 1),
                            )
                        ot = oc * (O_CHUNK // P) + os_
                        nc.scalar.activation(
                            y_next[:, ot, :],
                            ps[:],
                            mybir.ActivationFunctionType.Relu,
                            scale=SCALE,
                        )
                y_cur = y_next

            # readout: out[o, b] = sum_h beta[h, o] * y[h, b], scaled by 1/H
            for oc in range(OUT // O_CHUNK):
                b_sb = wpool.tile([P, KI, O_CHUNK], f16, tag="w")
                nc.sync.dma_start(
                    b_sb[:], beta_t[:, :, oc * O_CHUNK : (oc + 1) * O_CHUNK]
                )
                for os_ in range(O_CHUNK // P):
                    ps = pspool.tile([P, B], f32, tag="ps")
                    for ki in range(KI):
                        nc.tensor.matmul(
                            ps[:],
                            lhsT=b_sb[:, ki, os_ * P : (os_ + 1) * P],
                            rhs=y_cur[:, ki, :],
                            start=(ki == 0),
                            stop=(ki == KI - 1),
                        )
                    o_sb = opool.tile([P, B], f32, tag="o")
                    nc.scalar.activation(
                        o_sb[:],
                        ps[:],
                        mybir.ActivationFunctionType.Copy,
                        scale=OUT_SCALE,
                    )
                    nc.sync.dma_start(
                        out_t[:, oc * (O_CHUNK // P) + os_, :], o_sb[:]
                    )

        if loop:
            loop_cm.__exit__(None, None, None)

    nc.compile()
    _batch_pe_sems(nc)
    _BUILD_CACHE[key] = nc
    return nc


def _batch_pe_sems(nc):
    """Keep the PE engine-sem update only on stop (accumulation-group-final)
    matmuls; remap all waits on that sem conservatively (round up to the next
    kept updater) and rescale the loop-boundary add/sub bookkeeping.

    Rationale: at 8 busy cores the per-MM EVT_SEM write is exposed on the PE
    issue path (~26 ns/MM measured); one update per 16-MM PSUM group removes
    it. Waiters only ever move later (to a group boundary at most 15 MMs
    ahead, ~3.4 us), which the 8-bank PSUM / 3-buf weight pipelines absorb.
    """
    import bass_rust

    f = nc.m.functions[0]
    # Identify the PE engine sem from any matmul update.
    pe_sem = None
    for b in f.blocks:
        for i in b.instructions:
            if type(i).__name__ == "InstMatmult" and i.sync_info:
                for u in i.sync_info.on_update:
                    pe_sem = u.ant_name
                    break
            if pe_sem:
                break
        if pe_sem:
            break
    if pe_sem is None:
        return

    # Pass 1: walk matmul updaters in program order; build old->new count map.
    old_to_new = {0: 0}
    old = new = 0
    body_old = {}  # block name -> updater count in that block
    for b in f.blocks:
        cnt = 0
        for i in b.instructions:
            if type(i).__name__ != "InstMatmult" or not i.sync_info:
                continue
            ups = [u for u in i.sync_info.on_update if u.ant_name == pe_sem]
            if not ups:
                continue
            cnt += 1
            old += 1
            keep = bool(i.stop_tensor_calc)
            if keep:
                new += 1
                old_to_new[old] = new
            else:
                old_to_new[old] = new + 1  # wait for next kept updater
        body_old[b.name] = cnt

    # Pass 2: strip non-stop updates; count kept per block.
    body_new = {}
    for b in f.blocks:
        kept = 0
        for i in b.instructions:
            if type(i).__name__ != "InstMatmult" or not i.sync_info:
                continue
            si = i.sync_info
            ups = [u for u in si.on_update if u.ant_name == pe_sem]
            if not ups:
                continue
            if i.stop_tensor_calc:
                kept += 1
            else:
                i.sync_info = bass_rust.SyncInfo(
                    on_wait=list(si.on_wait),
                    on_update=[u for u in si.on_update if u.ant_name != pe_sem],
                )
        body_new[b.name] = kept

    per_iter_old = max(body_old.values())  # the loop body's updater count
    per_iter_new = {body_old[k]: body_new[k] for k in body_old if body_old[k]}

    # Pass 3: remap waits everywhere; rescale boundary add/sub event sems.
    for b in f.blocks:
        for i in b.instructions:
            si = i.sync_info
            if si is None:
                continue
            changed = False
            new_waits = []
            for w in si.on_wait:
                if w.ant_name == pe_sem and w.wait_reg is None and w.wait_value > 0:
                    v = old_to_new[w.wait_value]
                    new_waits.append(
                        bass_rust.SyncWait(
                            sync_type=w.sync_type,
                            id=w.id,
                            ant_name=w.ant_name,
                            wait_mode=w.wait_mode,
                            wait_value=v,
                            wait_reg=None,
                        )
                    )
                    changed = True
                else:
                    new_waits.append(w)
            new_ups = []
            for u in si.on_update:
                if (
                    u.ant_name == pe_sem
                    and type(i).__name__ == "InstEventSemaphore"
                    and u.update_value in per_iter_new
                ):
                    new_ups.append(
                        bass_rust.SyncUpdate(
                            sync_type=u.sync_type,
                            id=u.id,
                            ant_name=u.ant_name,
                            update_mode=u.update_mode,
                            update_value=per_iter_new[u.update_value],
                            update_reg=u.update_reg,
                        )
                    )
                    changed = True
                else:
                    new_ups.append(u)
            if changed:
                i.sync_info = bass_rust.SyncInfo(on_wait=new_waits, on_update=new_ups)


def _prep_in_maps(x, W, beta):
    import ml_dtypes

    bf16 = ml_dtypes.bfloat16
    x = np.asarray(x, dtype=np.float32)
    W = np.asarray(W, dtype=np.float32)
    beta = np.asarray(beta, dtype=np.float32)
    xt = np.ascontiguousarray(x.T.astype(bf16))  # [H, B_TOTAL]
    wt = np.ascontiguousarray(W.transpose(0, 2, 1).astype(bf16))  # [L, H(i), H(o)]
    beta = np.ascontiguousarray(beta.astype(bf16))
    return [
        {"xt": np.ascontiguousarray(xt[:, c * B : (c + 1) * B]), "wt": wt, "beta": beta}
        for c in range(N_CORES)
    ]


def kernel(x, W, beta):
    from concourse.bass_utils import run_bass_kernel_spmd

    nc = _build()
    in_maps = _prep_in_maps(x, W, beta)
    res = run_bass_kernel_spmd(nc, in_maps, core_ids=list(range(N_CORES)))
    outs = [r["out"] for r in res.results]  # each [OUT, B] channel-major
    return np.concatenate([o.T for o in outs], axis=0).astype(np.float32)

